# revision 1
# baseline (speedup 1.0000x reference)
"""Tensor-parallel causal MHA kernel for 8 Trainium2 NeuronCores.

Problem: B=4, L=2048, D=1024, H=16 heads (hd=64), f32, causal softmax.

Sharding: batch-DP x head-TP. Core c handles batch b=c//2 and head group
g=c%2 (8 heads = 512 feature dims). Each core computes its QKV column
shard, attention for its 8 heads over its batch, and a row-shard partial
O-projection. Host sums the two partials per batch and adds bo.

Per-core pipeline (feature-transposed layout to avoid all transposes of
intermediates):
  x^T (PE transpose)  ->  Q^T/K^T = Wq/Wk^T chunks @ x^T  (feature-major)
                          V       = x^T chunks @ Wv       (token-major,
                                    with a ones column per head for the
                                    softmax denominator)
  scores^T[k,q] = K^T.T @ Q^T  (per head, causal blocks only)
  att = exp(scores^T)  (no max subtraction: scores are O(5) bounded)
  diagonal blocks masked via affine_select
  AV^T[d+1,q] = V'.T @ att  accumulated over key blocks; row d is the
                softmax denominator l[q]
  normalize: att_out^T = AV^T[0:64] * broadcast(1/l)  (PE ones-broadcast)
  out[t,o] += att_out^T chunks.T @ Wo chunks  (partial; host adds pairs)

All heavy matmuls run in fp32r (fp32 with 12-bit mantissa, full PE rate).
Weights are pre-rounded to fp32r on the host; intermediates are rounded
by writing engine outputs with an fp32r view.
"""

import sys

if "/opt/trn_rl_repo" not in sys.path:
    sys.path.insert(0, "/opt/trn_rl_repo")

import numpy as np

import concourse.bass as bass
import concourse.tile as tile
from concourse import bacc, mybir
from concourse.bass_utils import run_bass_kernel_spmd
from concourse.masks import make_identity

F32 = mybir.dt.float32
F32R = mybir.dt.float32r
EXP = mybir.ActivationFunctionType.Exp
IDENT = mybir.ActivationFunctionType.Identity
COPY = mybir.ActivationFunctionType.Copy

N_CORES = 8


def round_f32r(a: np.ndarray) -> np.ndarray:
    """Round fp32 values to the fp32r grid (12-bit mantissa, round half up)."""
    u = np.ascontiguousarray(a, dtype=np.float32).view(np.uint32)
    r = ((u.astype(np.uint64) + 0x800) & 0xFFFFF000).astype(np.uint32)
    return r.view(np.float32)


def build_program(L=2048, D=1024, HPC=8, hd=64, repeat=1, phase_a_only=False,
                  with_bias=True, att_bf16=True):
    """Build the per-core SPMD Bass program. Returns the compiled Bacc.

    repeat>1 duplicates the whole pipeline (for timing: the wall-clock delta
    between repeat=K and repeat=1 isolates on-device execution time).
    """
    DQ = HPC * hd                # per-core head dims (columns of the shard)
    SL = 512                     # slab width (queries per attention slab)
    NS = L // SL                 # slabs
    TSUB = SL // 128             # 128-row tiles per slab
    DIN = D // 128               # contraction chunks
    DQT = DQ // 128              # 128-dim tiles of the shard
    HPP = 128 // hd              # heads per 128-dim tile (2)
    VW = hd + 1                  # V' width per head (ones column appended)

    nc = bacc.Bacc("TRN2", target_bir_lowering=False, debug=False)
    # att and V' share a dtype (matmul forbids mixing f32r with bf16)
    av_dt = mybir.dt.bfloat16 if att_bf16 else F32R

    x_d = nc.dram_tensor("x", [L, D], F32, kind="ExternalInput")
    wq_d = nc.dram_tensor("wq", [D, DQ], F32R, kind="ExternalInput")
    wk_d = nc.dram_tensor("wk", [D, DQ], F32R, kind="ExternalInput")
    wv_d = nc.dram_tensor("wv", [D, DQ], F32R, kind="ExternalInput")
    wo_d = nc.dram_tensor("wo", [DQ, D], F32R, kind="ExternalInput")
    bq_d = nc.dram_tensor("bq", [DQ], F32, kind="ExternalInput")   # pre-scaled
    bk_d = nc.dram_tensor("bk", [DQ], F32, kind="ExternalInput")
    bv_d = nc.dram_tensor("bv", [1, DQ], F32R, kind="ExternalInput")
    # additive causal masks (0 / -1e30) for the TSUB diagonal alignments,
    # plus an fp32r identity used to add them into score PSUM tiles
    mask_d = nc.dram_tensor("mask", [TSUB, 128, SL], F32R, kind="ExternalInput")
    idr_d = nc.dram_tensor("idr", [128, 128], F32R, kind="ExternalInput")
    out_d = nc.dram_tensor("out", [L, D], F32, kind="ExternalOutput")

    with tile.TileContext(nc) as tc:
        with (
            tc.tile_pool(name="persist", bufs=1) as persist,
            tc.tile_pool(name="consts", bufs=1) as consts,
        ):
            # persistent activations
            qt = persist.tile([128, DQT, L], F32, tag="qt")
            kt = persist.tile([128, DQT, L], F32, tag="kt")
            vt = persist.tile([128, L // 128, HPC, VW], av_dt, tag="vt")

            ident = consts.tile([128, 128], F32, tag="ident")
            ones_sc = consts.tile([128, 128], F32, tag="ones_sc")
            ones_k = consts.tile([1, 128], F32R, tag="ones_k")
            ones_b = consts.tile([1, hd], F32R, tag="ones_b")
            bq_sb = consts.tile([128, DQT], F32, tag="bq")
            bk_sb = consts.tile([128, DQT], F32, tag="bk")
            bv_sb = consts.tile([1, DQ], F32R, tag="bv")
            mask_sb = consts.tile([128, TSUB, SL], F32R, tag="mask")
            idr_sb = consts.tile([128, 128], F32R, tag="idr")
            # zero-padded moving-Q staging for K=128 score matmuls:
            # [parity][buffer]; rows of the *other* head parity stay zero
            zero_sc = consts.tile([hd, SL], F32, tag="zero_sc")
            stq = [[consts.tile([128, SL], F32R, tag=f"stq{p}{b}", name=f"stq{p}{b}")
                    for b in range(2)] for p in range(HPP)]

            make_identity(nc, ident[:])
            # fp32r constants must come from fp32r-writing producers (ACT)
            nc.gpsimd.memset(zero_sc[:], 0.0)
            for p in range(HPP):
                for b in range(2):
                    nc.scalar.activation(
                        stq[p][b][(1 - p) * hd : (2 - p) * hd, :],
                        zero_sc[:], COPY,
                    )
            nc.gpsimd.memset(ones_sc[:], 1.0)
            nc.scalar.activation(ones_k[:], ones_sc[0:1, :], COPY)
            nc.scalar.activation(ones_b[:], ones_sc[0:1, 0:hd], COPY)
            nc.scalar.activation(
                vt[:, :, :, hd],
                ones_sc[:].rearrange("p (a b) -> p a b", a=L // 128)[:, :, 0:HPC],
                COPY,
            )
            nc.sync.dma_start(bq_sb[:], bq_d[:].rearrange("(c p) -> p c", p=128))
            nc.sync.dma_start(bk_sb[:], bk_d[:].rearrange("(c p) -> p c", p=128))
            nc.sync.dma_start(bv_sb[:], bv_d[:])
            nc.sync.dma_start(mask_sb[:], mask_d[:].rearrange("t p q -> p t q"))
            nc.sync.dma_start(idr_sb[:], idr_d[:])

            # ---------------- Phase A: x^T and projections ----------------
            def one_pass():
              with (
                tc.tile_pool(name="wqkv", bufs=1) as wpool,
                tc.tile_pool(name="xa", bufs=4) as xa_pool,
                tc.tile_pool(name="xt", bufs=1) as xt_pool,
                tc.tile_pool(name="pxt", bufs=2, space="PSUM") as pxt_pool,
                tc.tile_pool(name="pproj", bufs=2, space="PSUM") as pproj_pool,
            ):
                wq_sb = wpool.tile([128, DIN, DQ], F32R, tag="wq")
                wk_sb = wpool.tile([128, DIN, DQ], F32R, tag="wk")
                wv_sb = wpool.tile([128, DIN, DQ], F32R, tag="wv")
                nc.sync.dma_start(
                    wq_sb[:], wq_d[:].rearrange("(c p) d -> p c d", p=128)
                )
                nc.sync.dma_start(
                    wk_sb[:], wk_d[:].rearrange("(c p) d -> p c d", p=128)
                )
                nc.sync.dma_start(
                    wv_sb[:], wv_d[:].rearrange("(c p) d -> p c d", p=128)
                )

                for s in range(NS):
                    xa = []
                    for ts in range(TSUB):
                        t = xa_pool.tile([128, D], F32, tag="xa")
                        nc.sync.dma_start(
                            t[:], x_d[s * SL + ts * 128 : s * SL + (ts + 1) * 128, :]
                        )
                        xa.append(t)
                    xt = xt_pool.tile([128, DIN, SL], F32, tag="xt")
                    for dc in range(DIN):
                        pxt = pxt_pool.tile([128, SL], F32, tag="pxt")
                        for ts in range(TSUB):
                            nc.tensor.transpose(
                                pxt[:, ts * 128 : (ts + 1) * 128],
                                xa[ts][:, dc * 128 : (dc + 1) * 128],
                                ident[:],
                            )
                        nc.scalar.activation(xt[:, dc, :].bitcast(F32R), pxt[:], COPY)

                    # Q^T and K^T (feature-major)
                    for w_sb, dst, scale, b_sb in (
                        (wq_sb, qt, 1.0 / np.sqrt(hd), bq_sb),
                        (wk_sb, kt, 1.0, bk_sb),
                    ):
                        for i in range(DQT):
                            pq = pproj_pool.tile([128, SL], F32, tag="pproj")
                            for dc in range(DIN):
                                nc.tensor.matmul(
                                    pq[:],
                                    w_sb[:, dc, i * 128 : (i + 1) * 128],
                                    xt[:, dc, :].bitcast(F32R),
                                    start=(dc == 0),
                                    stop=(dc == DIN - 1),
                                )
                            nc.scalar.activation(
                                dst[:, i, s * SL : (s + 1) * SL].bitcast(F32R),
                                pq[:],
                                IDENT,
                                bias=b_sb[:, i : i + 1],
                                scale=float(scale),
                            )

                    # V (token-major) with bias row
                    for ts in range(TSUB):
                        pv = pproj_pool.tile([128, DQ], F32, tag="pproj")
                        for dc in range(DIN):
                            nc.tensor.matmul(
                                pv[:, 0:DQ],
                                xt[:, dc, ts * 128 : (ts + 1) * 128].bitcast(F32R),
                                wv_sb[:, dc, :],
                                start=(dc == 0),
                                stop=(dc == DIN - 1) and not with_bias,
                            )
                        if with_bias:
                            nc.tensor.matmul(
                                pv[:, 0:DQ], ones_k[:].bitcast(F32R), bv_sb[:],
                                start=False, stop=True,
                            )
                        tci = s * TSUB + ts
                        nc.scalar.activation(
                            vt[:, tci, :, 0:hd],
                            pv[:, 0:DQ].rearrange("p (h c) -> p h c", c=hd),
                            COPY,
                        )

              if phase_a_only:
                  # timing attribution: dump projections instead of attention
                  with tc.tile_pool(name="dump", bufs=2) as dump_pool:
                      for i in range(DQT):
                          for s in range(NS):
                              dt_ = dump_pool.tile([128, SL], F32, tag="dump")
                              nc.vector.tensor_copy(dt_[:], qt[:, i, s * SL : (s + 1) * SL])
                              nc.sync.dma_start(
                                  out_d[(i * NS + s) * 128 : (i * NS + s + 1) * 128, 0:SL],
                                  dt_[:],
                              )
                  return

              # ---------------- Phase B: attention + O projection ----------------
              with (
                  tc.tile_pool(name="wob", bufs=1) as wob_pool,
                  tc.tile_pool(name="att", bufs=4) as att_pool,
                  tc.tile_pool(name="attoT", bufs=2) as attoT_pool,
                  tc.tile_pool(name="outsb", bufs=4) as out_pool,
                  tc.tile_pool(name="recip", bufs=2) as recip_pool,
                  tc.tile_pool(name="pscore", bufs=2, space="PSUM") as pscore_pool,
                  tc.tile_pool(name="pav", bufs=3, space="PSUM") as pav_pool,
                  tc.tile_pool(name="pout", bufs=1, space="PSUM") as pout_pool,
              ):
                  wo_sb = wob_pool.tile([128, DQT, D], F32R, tag="wo")
                  nc.sync.dma_start(
                      wo_sb[:], wo_d[:].rearrange("(c p) d -> p c d", p=128)
                  )

                  att_dt = mybir.dt.bfloat16 if att_bf16 else F32R

                  # deferred-op queue: AV matmuls (and head epilogues) trail
                  # the score/exp stream by one pair so the PE never sits
                  # waiting on the exp that feeds the next AV
                  stream = []

                  def emit_av(pav, kb, att_ap, h, nkb):
                      nc.tensor.matmul(
                          pav[:],
                          vt[:, kb, h, :],
                          att_ap,
                          start=(kb == 0),
                          stop=(kb == nkb - 1),
                      )

                  def emit_norm(pav, attoT, pr, ci):
                      # normalize by the ones-column row (fp32r views come
                      # from bit-copy DMAs; ACT stays exp-only)
                      recip = recip_pool.tile([1, SL], F32, tag="recip")
                      nc.vector.reciprocal(recip[:], pav[hd : hd + 1, :])
                      recip_r = recip_pool.tile([1, SL], F32R, tag="recip_r")
                      nc.sync.dma_start(recip_r[:], recip[:].bitcast(F32R))
                      pbc = pout_pool.tile([hd, SL], F32, tag="pout")
                      nc.tensor.matmul(pbc[:], ones_b[:], recip_r[:])
                      bc_sb = recip_pool.tile([hd, SL], F32, tag="bc_sb")
                      nc.vector.tensor_copy(bc_sb[:], pbc[:])
                      norm = recip_pool.tile([hd, SL], F32, tag="norm")
                      nc.vector.tensor_mul(norm[:], pav[0:hd, :], bc_sb[:])
                      nc.sync.dma_start(
                          attoT[pr : pr + hd, ci, :].bitcast(F32R),
                          norm[:].bitcast(F32R),
                      )

                  def flush(keep):
                      while len(stream) > keep:
                          op = stream.pop(0)
                          op[0](*op[1:])

                  heads = [(qs, h) for qs in range(NS) for h in range(HPC)]

                  def issue_stage(gi):
                      sqs, sh = heads[gi]
                      spr = (sh % HPP) * hd
                      sq = stq[sh % HPP][(gi // HPP) % 2]
                      nc.sync.dma_start(
                          sq[spr : spr + hd, :],
                          qt[spr : spr + hd, sh // HPP,
                             sqs * SL : (sqs + 1) * SL].bitcast(F32R),
                      )
                      return sq

                  staged = issue_stage(0)
                  for qs in range(NS):
                      attoT = attoT_pool.tile([128, DQT, SL], F32, tag="attoT")
                      nkb = (qs + 1) * TSUB
                      for h in range(HPC):
                          pr = (h % HPP) * hd      # partition offset in pair tile
                          ci = h // HPP            # which DQT tile
                          gi = qs * HPC + h
                          sq = staged
                          if gi + 1 < len(heads):
                              staged = issue_stage(gi + 1)
                          pav = pav_pool.tile([hd + 1, SL], F32, tag="pav")
                          for k0 in range(0, nkb, 2):
                              kbs = [kb for kb in (k0, k0 + 1) if kb < nkb]
                              ps = pscore_pool.tile([128, 2, SL], F32, tag="pscore")
                              for j, kb in enumerate(kbs):
                                  diag = kb >= qs * TSUB
                                  nc.tensor.matmul(
                                      ps[:, j, :],
                                      kt[:, ci, kb * 128 : (kb + 1) * 128].bitcast(F32R),
                                      sq[:],
                                      start=True,
                                      stop=not diag,
                                  )
                                  if diag:
                                      # add the triangular -1e30 mask
                                      nc.tensor.matmul(
                                          ps[:, j, :],
                                          idr_sb[:],
                                          mask_sb[:, kb - qs * TSUB, :],
                                          start=False,
                                          stop=True,
                                      )
                              att = att_pool.tile([128, 2, SL], att_dt, tag="att")
                              nc.scalar.activation(
                                  att[:, 0 : len(kbs), :],
                                  ps[:, 0 : len(kbs), :],
                                  EXP,
                              )
                              for j, kb in enumerate(kbs):
                                  stream.append((emit_av, pav, kb, att[:, j, :], h, nkb))
                                  flush(4)
                          stream.append((emit_norm, pav, attoT, pr, ci))

                      flush(0)  # attoT complete before the O projection reads it

                      # O projection for this slab (partial over the head shard)
                      for ts in range(TSUB):
                          for ob in range(D // SL):
                              po = pout_pool.tile([128, SL], F32, tag="pout")
                              for c in range(DQT):
                                  nc.tensor.matmul(
                                      po[:],
                                      attoT[:, c, ts * 128 : (ts + 1) * 128].bitcast(F32R),
                                      wo_sb[:, c, ob * SL : (ob + 1) * SL],
                                      start=(c == 0),
                                      stop=(c == DQT - 1),
                                  )
                              osb = out_pool.tile([128, SL], F32, tag="outsb")
                              nc.vector.tensor_copy(osb[:], po[:])
                              nc.sync.dma_start(
                                  out_d[
                                      qs * SL + ts * 128 : qs * SL + (ts + 1) * 128,
                                      ob * SL : (ob + 1) * SL,
                                  ],
                                  osb[:],
                              )

            for _rep in range(repeat):
                one_pass()

    nc.compile()
    return nc


_PROGRAMS = {}


def _get_program(with_bias=True):
    if with_bias not in _PROGRAMS:
        _PROGRAMS[with_bias] = build_program(with_bias=with_bias)
    return _PROGRAMS[with_bias]


def make_masks(SL=512, TSUB=4):
    """Additive causal masks for the TSUB diagonal alignments + fp32r identity."""
    col = np.arange(SL)[None, :]
    p = np.arange(128)[:, None]
    masks = np.zeros((TSUB, 128, SL), np.float32)
    for j in range(TSUB):
        masks[j] = np.where(col - j * 128 - p >= 0, 0.0, -1e30)
    return round_f32r(masks), round_f32r(np.eye(128, dtype=np.float32))


def _shard_inputs(x, Wq, bq, Wk, bk, Wv, bv, Wo, bo):
    """Build the 8 per-core input maps."""
    HIDDEN = Wq.shape[0]
    M = 2                     # head groups (tensor-parallel degree per batch)
    DQ = HIDDEN // M
    hd = 64
    masks, idr = make_masks()
    in_maps = []
    for c in range(N_CORES):
        b = c // M
        g = c % M
        cols = slice(g * DQ, (g + 1) * DQ)
        in_maps.append(
            {
                "x": np.ascontiguousarray(x[b]),
                "mask": masks,
                "idr": idr,
                "wq": round_f32r(Wq[:, cols]),
                "wk": round_f32r(Wk[:, cols]),
                "wv": round_f32r(Wv[:, cols]),
                "wo": round_f32r(Wo[cols, :]),
                "bq": np.ascontiguousarray(bq[cols] / np.float32(np.sqrt(hd))),
                "bk": np.ascontiguousarray(bk[cols]),
                "bv": round_f32r(bv[cols])[None, :],
            }
        )
    return in_maps


def kernel(**inputs) -> np.ndarray:
    x = np.asarray(inputs["x"], dtype=np.float32)
    B, L, D = x.shape
    with_bias = any(
        np.any(np.asarray(inputs[k])) for k in ("bq", "bk", "bv")
    )
    nc = _get_program(with_bias)
    in_maps = _shard_inputs(
        x,
        np.asarray(inputs["Wq"], np.float32), np.asarray(inputs["bq"], np.float32),
        np.asarray(inputs["Wk"], np.float32), np.asarray(inputs["bk"], np.float32),
        np.asarray(inputs["Wv"], np.float32), np.asarray(inputs["bv"], np.float32),
        np.asarray(inputs["Wo"], np.float32), np.asarray(inputs["bo"], np.float32),
    )
    res = run_bass_kernel_spmd(nc, in_maps, list(range(N_CORES)))
    bo = np.asarray(inputs["bo"], np.float32)
    out = np.empty((B, L, D), np.float32)
    for b in range(B):
        out[b] = res.results[2 * b]["out"] + res.results[2 * b + 1]["out"] + bo
    return out

